# revision 26
# baseline (speedup 1.0000x reference)
"""Trainium2 Bass kernel for nn_FACoef.

Computes, for each batch b of x (B, 512, 512):
    out[b] = sum_{i<3, j<3} coef[i,j] * sum_elems((x_b^(i+2)) ** (j+1)) / (N*N)^(i+j+2)

Strategy (pure data parallel, 8 batches per core on 8 NeuronCores):
  Work with y = x^T.  y^k = (x^k)^T, and the elementwise power-sums are
  transpose invariant, so we compute the chain y2 = y@y, y3 = y@y2,
  y4 = y@y3 on the PE with the *natural-layout* x as the stationary
  operand (lhsT = (y)^T = x), needing only one PE transpose of x per
  batch to seed the chain's first rhs.

  Matmuls run in float32r (single-pass FP22 multiply, full PE rate).
  Per result matrix (128x2048 row-block-major layout):
    - ScalarE: Copy psum->sbuf with fused accum  -> s1 partials (+ rhs copy)
    - ScalarE: Square (first RA blocks) + accum  -> s2a partials, t2a
    - VectorE: square (rest) via scalar_tensor_tensor + accum -> s2b, t2b
    - VectorE: tensor_tensor_reduce t2*y + accum -> s3 partials
  Per-partition partial sums land in accumulator tiles, DMA'd out; the
  host reduces partitions and applies coef/norm in float64.
"""

import numpy as np

import concourse.bacc as bacc
import concourse.bass as bass
import concourse.mybir as mybir
import concourse.tile as tile
from concourse.bass_utils import run_bass_kernel_spmd

N = 512
RB = 4  # row blocks of 128
BPC = 8  # batches per core
NCORES = 8
ROWS = 3
COLS = 3
RA = 2  # r-blocks of the square pass done on ScalarE (rest on VectorE)

FP32 = mybir.dt.float32
FP32R = mybir.dt.float32r
AF = mybir.ActivationFunctionType
ALU = mybir.AluOpType


def build_nc():
    nc = bacc.Bacc(None, target_bir_lowering=False)
    x_ext = nc.declare_dram_parameter("x", [BPC, N, N], FP32, isOutput=False)
    xt_ext = nc.declare_dram_parameter("xt", [BPC, N, N], FP32, isOutput=False)
    # acc_a: per (batch, mat): [s1, s2a];  acc_d: [s2b, s3a, s3b]
    acc_a_ext = nc.declare_dram_parameter("acc_a", [128, BPC * ROWS * 2], FP32, isOutput=True)
    acc_d_ext = nc.declare_dram_parameter("acc_d", [128, BPC * ROWS * 3], FP32, isOutput=True)

    with tile.TileContext(nc) as tc:
        with (
            tc.tile_pool(name="xpool", bufs=16) as xpool,
            tc.tile_pool(name="ycpool", bufs=16) as ycpool,
            tc.tile_pool(name="ypool", bufs=9) as ypool,
            tc.tile_pool(name="tpool", bufs=3) as tpool,
            tc.tile_pool(name="accpool", bufs=1) as accpool,
            tc.tile_pool(name="ps", bufs=2, space="PSUM") as pspool,
        ):
            acc_a = accpool.tile([128, BPC * ROWS * 2], FP32)
            acc_d = accpool.tile([128, BPC * ROWS * 3], FP32)

            def load_batch(b):
                # per-row-block chunk DMAs (one HW queue each, fine-grained
                # deps so kk=0 matmuls can start after the first chunks land)
                sbx_c, yc_c = [], []
                for kk in range(RB):
                    sc = xpool.tile([128, N], FP32R, tag="sbx")
                    nc.sync.dma_start(
                        out=sc,
                        in_=x_ext[b, 128 * kk : 128 * (kk + 1), :].bitcast(FP32R),
                    )
                    yc = ycpool.tile([128, N], FP32R, tag="yc")
                    nc.sync.dma_start(
                        out=yc,
                        in_=xt_ext[b, 128 * kk : 128 * (kk + 1), :].bitcast(FP32R),
                    )
                    sbx_c.append(sc)
                    yc_c.append(yc)
                return sbx_c, yc_c

            def chain_step(sbx_c, ycur, ci, first):
                """One matmul group + elementwise power-sums; returns new ycur.

                first=True: ycur is a list of 4 chunk tiles (DMA-fed) and the
                kk loop goes outermost so compute starts on the first chunk.
                Otherwise ycur is a (128, RB*N) tile from the previous step.
                """
                psY = pspool.tile([128, RB * N], FP32, tag="ps")
                if first:
                    for kk in range(RB):
                        for m in range(RB):
                            nc.tensor.matmul(
                                psY[:, m * N : (m + 1) * N],
                                lhsT=sbx_c[kk][:, 128 * m : 128 * (m + 1)],
                                rhs=ycur[kk][:, :],
                                start=(kk == 0),
                                stop=(kk == RB - 1),
                            )
                else:
                    for m in range(RB):
                        for kk in range(RB):
                            nc.tensor.matmul(
                                psY[:, m * N : (m + 1) * N],
                                lhsT=sbx_c[kk][:, 128 * m : 128 * (m + 1)],
                                rhs=ycur[:, kk * N : (kk + 1) * N],
                                start=(kk == 0),
                                stop=(kk == RB - 1),
                            )
                ysb = ypool.tile([128, RB * N], FP32R, tag="y")
                # copy psum->sbuf + s1 partials
                nc.scalar.activation(
                    ysb, psY, AF.Copy, accum_out=acc_a[:, 2 * ci + 1 : 2 * ci + 2]
                )
                # squares: ScalarE on first RA blocks, VectorE on the rest
                t2a = tpool.tile([128, RA * N], FP32, tag="t2a")
                nc.scalar.activation(
                    t2a,
                    ysb[:, : RA * N].bitcast(FP32),
                    AF.Square,
                    accum_out=acc_a[:, 2 * ci : 2 * ci + 1],
                )
                t2b = tpool.tile([128, (RB - RA) * N], FP32, tag="t2b")
                nc.vector.scalar_tensor_tensor(
                    out=t2b,
                    in0=ysb[:, RA * N :].bitcast(FP32),
                    scalar=1.0,
                    in1=ysb[:, RA * N :].bitcast(FP32),
                    op0=ALU.mult,
                    op1=ALU.mult,
                    accum_out=acc_d[:, 3 * ci : 3 * ci + 1],
                )
                # cubes: t3 = t2 * y with fused reduction
                t3a = tpool.tile([128, RA * N], FP32, tag="t3a")
                nc.vector.affine_mul_reduce(
                    out=t3a,
                    accum_out=acc_d[:, 3 * ci + 1 : 3 * ci + 2],
                    in0=t2a,
                    in1=ysb[:, : RA * N].bitcast(FP32),
                    scale=1.0,
                    bias=0.0,
                )
                t3b = tpool.tile([128, (RB - RA) * N], FP32, tag="t3b")
                nc.vector.affine_mul_reduce(
                    out=t3b,
                    accum_out=acc_d[:, 3 * ci + 2 : 3 * ci + 3],
                    in0=t2b,
                    in1=ysb[:, RA * N :].bitcast(FP32),
                    scale=1.0,
                    bias=0.0,
                )
                return ysb

            # Software-pipelined batch pairs: alternate the two batches' chain
            # steps so each ACT copy hides under the other batch's matmuls and
            # the PE never idles (keeps HAM at full clock).  Loads are emitted
            # one pair ahead of compute.
            npairs = BPC // 2
            loaded = {0: (load_batch(0), load_batch(1))}
            for pair in range(npairs):
                ba, bb = 2 * pair, 2 * pair + 1
                (sbx_a, ycur_a), (sbx_b, ycur_b) = loaded.pop(pair)
                if pair + 1 < npairs:
                    loaded[pair + 1] = (
                        load_batch(2 * pair + 2),
                        load_batch(2 * pair + 3),
                    )
                for k in range(ROWS):
                    ycur_a = chain_step(sbx_a, ycur_a, ba * ROWS + k, k == 0)
                    ycur_b = chain_step(sbx_b, ycur_b, bb * ROWS + k, k == 0)

            nc.sync.dma_start(out=acc_a_ext[:, :], in_=acc_a)
            nc.sync.dma_start(out=acc_d_ext[:, :], in_=acc_d)

    # The fused-LDW fp32 matmul encoding only fits ONE sync wait.  Tile
    # emits conservative PE-self waits for PSUM slot reuse (WAW vs earlier
    # matmuls), but PE matmuls complete strictly in program order through
    # the single PSUM write port, so those waits are redundant — drop them.
    import os
    if os.environ.get("NO_STRIP_PE_WAITS", "") != "1":
        for bb in nc.m.functions[0].blocks:
            for ins in bb.instructions:
                if type(ins).__name__ == "InstMatmult":
                    si = ins.sync_info
                    own = {u.id for u in si.on_update}
                    kept = [w for w in si.on_wait if w.id not in own]
                    if len(kept) != len(si.on_wait):
                        si.on_wait = kept
                        ins.sync_info = si

    if os.environ.get("NO_STRIP_PE_INCS", "") != "1":
        _strip_unwaited_matmul_incs(nc)

    nc.finalize()
    return nc


def _strip_unwaited_matmul_incs(nc):
    """Each matmul's semaphore increment costs ~26ns of PE time.  Matmuls
    complete strictly in order, so only the ticks some wait references are
    needed: keep those increments (plus the final one) and renumber every
    wait on the PE semaphore accordingly."""
    all_ins = [ins for bb in nc.m.functions[0].blocks for ins in bb.instructions]
    mms = [i for i in all_ins if type(i).__name__ == "InstMatmult"]
    if not mms:
        return
    pe_sem_ids = set()
    for m in mms:
        for u in m.sync_info.on_update:
            if u.sync_type == "semaphore" and u.update_mode == "sem-inc":
                pe_sem_ids.add(u.id)
    if len(pe_sem_ids) != 1:
        return  # unexpected shape; leave untouched
    sem_id = pe_sem_ids.pop()
    # Any non-matmul instruction that also increments this sem would break
    # the renumbering — bail out in that case.
    for ins in all_ins:
        if type(ins).__name__ == "InstMatmult" or ins.sync_info is None:
            continue
        for u in ins.sync_info.on_update:
            if u.id == sem_id:
                return
    # Ticks: matmul #p (1-indexed in appearance order) completes -> sem = p.
    waited = set()
    for ins in all_ins:
        if ins.sync_info is None:
            continue
        for w in ins.sync_info.on_wait:
            if w.id == sem_id:
                waited.add(w.wait_value)
    keep = sorted(waited | {len(mms)})
    if any(v < 1 or v > len(mms) for v in keep):
        return
    rank = {v: i + 1 for i, v in enumerate(keep)}
    keep_set = set(keep)
    for p, m in enumerate(mms, start=1):
        si = m.sync_info
        if p not in keep_set:
            si.on_update = [u for u in si.on_update if u.id != sem_id]
            m.sync_info = si
    for ins in all_ins:
        si = ins.sync_info
        if si is None:
            continue
        changed = False
        new_waits = []
        for w in si.on_wait:
            if w.id == sem_id:
                w.wait_value = rank[w.wait_value]
                changed = True
            new_waits.append(w)
        if changed:
            si.on_wait = new_waits
            ins.sync_info = si


_NC_CACHE = None


def get_nc():
    global _NC_CACHE
    if _NC_CACHE is None:
        _NC_CACHE = build_nc()
    return _NC_CACHE


def combine_partials(acc_a, acc_d, coef, out, base):
    """Reduce per-partition partials and apply coef/norm in float64."""
    a = acc_a.astype(np.float64).sum(axis=0)  # (BPC*ROWS*2,)
    d = acc_d.astype(np.float64).sum(axis=0)  # (BPC*ROWS*3,)
    norm_pow = (
        np.arange(COLS)[None, :] + np.arange(ROWS)[:, None] + 2
    ).astype(np.float64)
    w = coef.astype(np.float64) / (float(N * N) ** norm_pow)  # (ROWS, COLS)
    for b in range(BPC):
        acc = 0.0
        for i in range(ROWS):
            ci = b * ROWS + i
            s1 = a[2 * ci + 1]
            s2 = a[2 * ci] + d[3 * ci]
            s3 = d[3 * ci + 1] + d[3 * ci + 2]
            acc += w[i, 0] * s1 + w[i, 1] * s2 + w[i, 2] * s3
        out[base + b] = acc


def kernel(x, coef):
    x = np.ascontiguousarray(x, dtype=np.float32)
    coef = np.asarray(coef, dtype=np.float32)
    B = x.shape[0]
    assert B == BPC * NCORES and x.shape[1:] == (N, N)

    nc = get_nc()
    xt = np.ascontiguousarray(x.transpose(0, 2, 1))
    in_maps = [
        {
            "x": x[c * BPC : (c + 1) * BPC],
            "xt": xt[c * BPC : (c + 1) * BPC],
        }
        for c in range(NCORES)
    ]
    res = run_bass_kernel_spmd(nc, in_maps, list(range(NCORES))).results

    out = np.zeros(B, dtype=np.float64)
    for c in range(NCORES):
        combine_partials(res[c]["acc_a"], res[c]["acc_d"], coef, out, c * BPC)
    return out.astype(np.float32)
